# revision 5
# baseline (speedup 1.0000x reference)
"""Trainium2 Bass kernel for a bidirectional cross-attention layer.

Per batch sample (one NeuronCore each, 8 samples / 8 cores):
    e  = seq_1 @ seq_2^T                     [L, L]
    P  = exp(e)            (no max-subtraction: |e| <~ 70 << fp32 overflow)
    seq_1_hat = diag(1/rowsum(P)) @ P   @ seq_2
    seq_2_hat = diag(1/colsum(P)) @ P^T @ seq_1

v2 design: single exp pass. P is materialized once in bf16; the transposed
orientation P2 = P^T needed by the seq_1_hat accumulation is produced by the
DMA XBAR transpose engine (idle otherwise) instead of recomputing scores and
re-exponentiating (which made the scalar engine the bottleneck in v1).
Matmul operands are fp16/bf16: halves instruction count (1024-wide moving
operands) and LDWEIGHTS cost. All input/output DMAs are single batched
transfers; input casts f32->fp16 ride gpsimd cast-DMAs.

Phase A (per i-block): score matmul -> exp -> P1[i,j] bf16; rowsum via one
DVE reduce; o2T accumulation matmul; XBAR transpose of the P1 block row
into P2. Phase B (per j-block): o1T accumulation from P2; colsum reduces
split DVE/ACT. Epilogues transpose accumulators back via XBAR and scale
with per-partition reciprocals.
"""

import os

os.environ.setdefault("MYCRO_LOCAL_CACHE", "1")

import numpy as np

import concourse.mybir as mybir
from concourse import bacc
from concourse.bass_utils import run_bass_kernel_spmd
from concourse.tile import TileContext

B, L, D = 8, 2048, 128
NBLK = L // 128  # 16 blocks of 128

F32 = mybir.dt.float32
F16 = mybir.dt.float16
BF16 = mybir.dt.bfloat16
AF = mybir.ActivationFunctionType
ALU = mybir.AluOpType
AX = mybir.AxisListType


def _build():
    nc = bacc.Bacc(
        "TRN2", target_bir_lowering=False, debug=False, enable_asserts=False
    )
    s1 = nc.dram_tensor("seq_1", [L, D], F32, kind="ExternalInput").ap()
    s2 = nc.dram_tensor("seq_2", [L, D], F32, kind="ExternalInput").ap()
    o1 = nc.dram_tensor("out1", [L, D], BF16, kind="ExternalOutput").ap()
    o2 = nc.dram_tensor("out2", [L, D], BF16, kind="ExternalOutput").ap()

    with TileContext(nc) as tc:
        with tc.tile_pool(name="big", bufs=1) as big:
            s1h = big.tile([128, L], F16, tag="s1h")  # [i%128, blk*128+d]
            s2h = big.tile([128, L], F16, tag="s2h")
            s1t = big.tile([128, L], F16, tag="s1t")  # [d, i]
            s2t = big.tile([128, L], F16, tag="s2t")  # [d, j]
            P1 = big.tile([128, NBLK * L], BF16, tag="P1")  # [i%128, ib*L+j]
            P2 = big.tile([128, NBLK * L], BF16, tag="P2")  # [j%128, jb*L+i]
            o2h = big.tile([128, L], BF16, tag="o2h")   # bf16 copy of acc0
            o1h = big.tile([128, L], BF16, tag="o1h")
            o2s = big.tile([128, L], BF16, tag="o2s")   # xbar'd [j%128, jb*128+d]
            o1s = big.tile([128, L], BF16, tag="o1s")
            ob2 = big.tile([128, L], BF16, tag="ob2")   # scaled, store staging
            ob1 = big.tile([128, L], BF16, tag="ob1")
            scr = big.tile([128, L], BF16, tag="scr")   # ACT colsum scratch
            rowsum = big.tile([128, NBLK], F32, tag="rowsum")
            colsum = big.tile([128, NBLK], F32, tag="colsum")
            rrow = big.tile([128, NBLK], F32, tag="rrow")
            rcol = big.tile([128, NBLK], F32, tag="rcol")

            # ---- preload: cast-DMA f32->fp16, then XBAR-transpose --------
            # s2 first: phase A needs the full s2t, but only per-block s1t.
            for dst, src in ((s2h, s2), (s1h, s1)):
                nc.gpsimd.dma_start(
                    dst.rearrange("p (b d) -> p b d", d=128),
                    src.rearrange("(b r) d -> r b d", r=128),
                )
            for dst, src in ((s2t, s2h), (s1t, s1h)):
                nc.sync.dma_start_transpose(
                    dst.rearrange("p (b r) -> p b r", r=128), src
                )

            P2_3d = P2.rearrange("p (b i) -> p b i", i=L)

            # ---- phase A: P1 tiles, rowsum, o2T accumulation, P2 xbar ----
            with (
                tc.tile_pool(name="acc0p", bufs=1, space="PSUM") as acc0p,
                tc.tile_pool(name="ep", bufs=2, space="PSUM") as ep,
            ):
                acc0 = acc0p.tile([128, L], F32, tag="acc0")
                for b in range(NBLK):
                    bsl = slice(b * 128, (b + 1) * 128)
                    for h in range(2):
                        jsl = slice(h * 1024, (h + 1) * 1024)
                        et = ep.tile([128, 1024], F32, tag="et")
                        for q in range(2):
                            # fp16 moving operand is capped at 512 wide
                            nc.tensor.matmul(
                                et[:, q * 512 : (q + 1) * 512],
                                lhsT=s1t[:, bsl],
                                rhs=s2t[:, h * 1024 + q * 512 : h * 1024 + (q + 1) * 512],
                                start=True, stop=True,
                            )
                        nc.scalar.activation(
                            P1[:, b * L + h * 1024 : b * L + (h + 1) * 1024],
                            et, AF.Exp,
                        )
                        for q in range(2):
                            # matmul out limited to one PSUM bank (512 f32)
                            csl = slice(h * 1024 + q * 512, h * 1024 + (q + 1) * 512)
                            nc.tensor.matmul(
                                acc0[:, csl], lhsT=s1h[:, bsl],
                                rhs=P1[:, b * L + h * 1024 + q * 512
                                       : b * L + h * 1024 + (q + 1) * 512],
                                start=(b == 0), stop=(b == NBLK - 1),
                            )
                    nc.vector.tensor_reduce(
                        rowsum[:, b : b + 1],
                        P1[:, b * L : (b + 1) * L],
                        axis=AX.X, op=ALU.add,
                    )
                    nc.sync.dma_start_transpose(
                        P2_3d[:, :, b * 128 : (b + 1) * 128],
                        P1[:, b * L : (b + 1) * L],
                    )
                nc.vector.reciprocal(rrow, rowsum)
                # boundary: drain acc0 so its PSUM banks free for acc1
                nc.scalar.copy(o2h, acc0)

            # ---- phase B: o1T accumulation + colsum + o2 epilogue --------
            with tc.tile_pool(name="acc1p", bufs=1, space="PSUM") as acc1p:
                acc1 = acc1p.tile([128, L], F32, tag="acc1")
                # o2 path: xbar early; per-block scale once colsum lands
                nc.sync.dma_start_transpose(
                    o2s.rearrange("p (b d) -> p b d", d=128), o2h
                )
                for b in range(NBLK):
                    bsl = slice(b * 128, (b + 1) * 128)
                    for c in range(4):
                        isl = slice(c * 512, (c + 1) * 512)
                        nc.tensor.matmul(
                            acc1[:, isl], lhsT=s2h[:, bsl],
                            rhs=P2[:, b * L + c * 512 : b * L + (c + 1) * 512],
                            start=(b == 0), stop=(b == NBLK - 1),
                        )
                    if b % 2 == 0:
                        nc.vector.tensor_reduce(
                            colsum[:, b : b + 1],
                            P2[:, b * L : (b + 1) * L],
                            axis=AX.X, op=ALU.add,
                        )
                    else:
                        nc.scalar.activation(
                            scr, P2[:, b * L : (b + 1) * L], AF.Copy,
                            accum_out=colsum[:, b : b + 1],
                        )
                    nc.vector.reciprocal(
                        rcol[:, b : b + 1], colsum[:, b : b + 1]
                    )
                    nc.vector.tensor_scalar_mul(
                        ob2[:, bsl], o2s[:, bsl], rcol[:, b : b + 1]
                    )
                nc.sync.dma_start(
                    o2.rearrange("(b r) d -> r b d", r=128),
                    ob2.rearrange("p (b d) -> p b d", d=128),
                )

                # ---- tail: o1 epilogue, chunked halves for overlap -------
                for h in range(2):
                    isl = slice(h * 1024, (h + 1) * 1024)
                    nc.scalar.copy(o1h[:, isl], acc1[:, isl])
                    nc.sync.dma_start_transpose(
                        o1s.rearrange("p (b d) -> p b d", d=128)[
                            :, h * 8 : (h + 1) * 8, :
                        ],
                        o1h[:, isl],
                    )
                    for k in range(h * 8, (h + 1) * 8):
                        nc.vector.tensor_scalar_mul(
                            ob1[:, k * 128 : (k + 1) * 128],
                            o1s[:, k * 128 : (k + 1) * 128],
                            rrow[:, k : k + 1],
                        )
                    nc.sync.dma_start(
                        o1.rearrange("(b r) d -> r b d", r=128)[
                            :, h * 8 : (h + 1) * 8, :
                        ],
                        ob1.rearrange("p (b d) -> p b d", d=128)[
                            :, h * 8 : (h + 1) * 8, :
                        ],
                    )

    nc.compile()
    return nc


_nc_cache = None


def _run(seq_1, seq_2, trace=False):
    global _nc_cache
    if _nc_cache is None:
        _nc_cache = _build()
    nc = _nc_cache
    seq_1 = np.ascontiguousarray(np.asarray(seq_1, dtype=np.float32))
    seq_2 = np.ascontiguousarray(np.asarray(seq_2, dtype=np.float32))
    in_maps = [{"seq_1": seq_1[b], "seq_2": seq_2[b]} for b in range(B)]
    res = run_bass_kernel_spmd(nc, in_maps, core_ids=list(range(B)), trace=trace)
    out1 = np.stack(
        [np.asarray(res.results[b]["out1"]).astype(np.float32) for b in range(B)]
    )
    out2 = np.stack(
        [np.asarray(res.results[b]["out2"]).astype(np.float32) for b in range(B)]
    )
    return (out1, out2), res


def kernel(seq_1, seq_2):
    return _run(seq_1, seq_2)[0]
